# revision 1
# baseline (speedup 1.0000x reference)
"""2-layer GraphSAGE (PyG SAGEConv, project=True, mean agg) on 8 trn2 NeuronCores.

Strategy (graph/data parallel, hardcoded for N=50000, E=800000, D=128, 8 cores):
  - Nodes sharded by contiguous ranges of 6250 (padded to 6272 = 49*128) per core.
  - Host preprocesses edges: sorted by (dst core, dst block, src half, src),
    padded so every (block, half) has a uniform chunk count across cores (SPMD).
  - Device per layer:
      * project own rows: p = relu(x @ WpT + bp)  -> fp16, AllGather into a
        replicated [50176,128] fp16 table in DRAM.
      * dma_gather (SWDGE) message rows from the table (two int16-indexed
        halves), 128 edges per chunk.
      * scatter via one-hot matmuls: aggT[k,d] += msg[e,k]^T @ onehot[e,d],
        onehot built on DVE with is_equal against an iota tile.
      * mean via per-dst invdeg multiply, then output matmuls + bias (+relu).
  - Layer-2 output rows are written per core and concatenated on host.
"""

import math
from contextlib import ExitStack

import numpy as np

import concourse.bacc as bacc
import concourse.bass as bass
import concourse.tile as tile
from concourse import library_config, mybir
from concourse.bass_utils import run_bass_kernel_spmd

P = 128
D = 128
CORES = 8
N_NODES = 50000
N_EDGES = 800000

AF = mybir.ActivationFunctionType
OP = mybir.AluOpType
dt = mybir.dt


def _plan(n_nodes, cores):
    nloc = n_nodes // cores
    assert nloc * cores == n_nodes
    nb = math.ceil(nloc / P)
    nloc_pad = nb * P
    npad = cores * nloc_pad
    nhalf = npad // 2
    assert nhalf < 32768, "dma_gather idx is int16"
    return nloc, nb, nloc_pad, npad, nhalf


def preprocess(edge_index, n_nodes, cores):
    """Returns per-core gather/scatter metadata + uniform chunk counts K0, K1."""
    nloc, nb, nloc_pad, npad, nhalf = _plan(n_nodes, cores)
    src = np.asarray(edge_index[0], dtype=np.int64)
    dst = np.asarray(edge_index[1], dtype=np.int64)
    E = src.shape[0]

    deg = np.bincount(dst, minlength=n_nodes).astype(np.float64)
    invdeg = (1.0 / np.maximum(deg, 1.0)).astype(np.float32)

    csrc = src // nloc
    r_src = csrc * nloc_pad + (src - csrc * nloc)  # padded row id of source
    half = (r_src >= nhalf).astype(np.int64)
    idx_in_half = (r_src - half * nhalf).astype(np.int64)

    cdst = dst // nloc
    ldst = dst - cdst * nloc
    blk = ldst // P
    dblk = ldst % P

    # sort edges by (dst core, dst block, src half, src row) — src order gives
    # the DMA engines ascending-address locality within each gather list
    order = np.lexsort((idx_in_half, half, blk, cdst))
    s_half = half[order]
    s_idx = idx_in_half[order]
    s_dblk = dblk[order]
    key = ((cdst[order] * nb + blk[order]) * 2 + s_half).astype(np.int64)

    counts = np.bincount(key, minlength=cores * nb * 2)
    starts = np.zeros(cores * nb * 2 + 1, dtype=np.int64)
    np.cumsum(counts, out=starts[1:])
    rank = np.arange(E, dtype=np.int64) - starts[key]

    cnt = counts.reshape(cores, nb, 2)
    K0 = max(1, int(math.ceil(cnt[:, :, 0].max() / P)))
    K1 = max(1, int(math.ceil(cnt[:, :, 1].max() / P)))

    # idx arrays: [cores, nb, K*P] int16 (pad = 0, harmless row gathered,
    # neutralized by dloc pad = 255 in the one-hot); dloc: [cores, nb, (K0+K1)*P]
    idx0 = np.zeros((cores, nb, K0 * P), dtype=np.int16)
    idx1 = np.zeros((cores, nb, K1 * P), dtype=np.int16)
    dloc = np.full((cores, nb, (K0 + K1) * P), 255.0, dtype=np.float16)

    core_k = key // (nb * 2)
    blk_k = (key // 2) % nb
    m0 = s_half == 0
    m1 = ~m0
    idx0[core_k[m0], blk_k[m0], rank[m0]] = s_idx[m0].astype(np.int16)
    idx1[core_k[m1], blk_k[m1], rank[m1]] = s_idx[m1].astype(np.int16)
    dloc[core_k[m0], blk_k[m0], rank[m0]] = s_dblk[m0].astype(np.float16)
    dloc[core_k[m1], blk_k[m1], K0 * P + rank[m1]] = s_dblk[m1].astype(np.float16)

    def wrap_idx(a):  # [nb, K*P] -> [128, nb*K*P//16] dma_gather layout
        flat = a.reshape(-1)
        w = flat.reshape(-1, 16).T  # [16, I/16]
        return np.tile(w, (8, 1)).copy()

    per_core = []
    for c in range(cores):
        dl = dloc[c].reshape(nb, K0 + K1, P).transpose(2, 0, 1).reshape(P, -1)
        inv = np.ones(nloc_pad, dtype=np.float32)
        inv[:nloc] = invdeg[c * nloc : (c + 1) * nloc]
        per_core.append(
            dict(
                idx0=wrap_idx(idx0[c]),
                idx1=wrap_idx(idx1[c]),
                dloc=np.ascontiguousarray(dl),
                invd=np.broadcast_to(inv[None, :], (P, nloc_pad)).copy(),
            )
        )
    return per_core, K0, K1, invdeg


def build_nc(n_nodes, cores, K0, K1, G, iters=1):
    nloc, nb, nloc_pad, npad, nhalf = _plan(n_nodes, cores)
    assert nb % G == 0
    ngroups = nb // G
    KT = K0 + K1

    nc = bacc.Bacc("TRN2", target_bir_lowering=False, debug=False, num_devices=cores)

    x_own = nc.dram_tensor("x_own", [nloc_pad, D], dt.float32, kind="ExternalInput").ap()
    idx0_d = nc.dram_tensor("idx0", [P, nb * K0 * P // 16], dt.int16, kind="ExternalInput").ap()
    idx1_d = nc.dram_tensor("idx1", [P, nb * K1 * P // 16], dt.int16, kind="ExternalInput").ap()
    dloc_d = nc.dram_tensor("dloc", [P, nb * KT], dt.float16, kind="ExternalInput").ap()
    invd_d = nc.dram_tensor("invd", [P, nloc_pad], dt.float32, kind="ExternalInput").ap()
    wdram = {
        n: nc.dram_tensor(n, [P, D], dt.float16, kind="ExternalInput").ap()
        for n in ["Wp1T", "Wl1T", "Wr1T", "Wp2T", "Wl2T", "Wr2T"]
    }
    bp1b_d = nc.dram_tensor("bp1b", [P, D], dt.float32, kind="ExternalInput").ap()
    bl1c_d = nc.dram_tensor("bl1c", [P, 1], dt.float32, kind="ExternalInput").ap()
    bp2b_d = nc.dram_tensor("bp2b", [P, D], dt.float32, kind="ExternalInput").ap()
    bl2b_d = nc.dram_tensor("bl2b", [P, D], dt.float32, kind="ExternalInput").ap()
    iota_d = nc.dram_tensor("iota", [P, P], dt.float16, kind="ExternalInput").ap()
    ident_d = nc.dram_tensor("ident", [P, P], dt.float16, kind="ExternalInput").ap()

    out_own = nc.dram_tensor("out_own", [nloc_pad, D], dt.float32, kind="ExternalOutput").ap()
    h1own = nc.dram_tensor("h1own", [nloc_pad, D], dt.float16).ap()
    h2own = nc.dram_tensor("h2own", [nloc_pad, D], dt.float16).ap()
    table1 = nc.dram_tensor("table1", [npad, D], dt.float16, addr_space="Shared").ap()
    table2 = nc.dram_tensor("table2", [npad, D], dt.float16, addr_space="Shared").ap()

    groups_all = [list(range(cores))]

    with tile.TileContext(nc) as tc, ExitStack() as ctx:
        const = ctx.enter_context(tc.tile_pool(name="const", bufs=1))
        persist = ctx.enter_context(tc.tile_pool(name="persist", bufs=1))
        stage_p = ctx.enter_context(tc.tile_pool(name="stage", bufs=2))
        work = ctx.enter_context(tc.tile_pool(name="work", bufs=3))
        ohp = ctx.enter_context(tc.tile_pool(name="oh", bufs=4))
        aggsb = ctx.enter_context(tc.tile_pool(name="aggsb", bufs=2))
        outp = ctx.enter_context(tc.tile_pool(name="outp", bufs=3))
        psum_agg = ctx.enter_context(tc.tile_pool(name="psum_agg", bufs=4, space="PSUM"))
        psum_mm = ctx.enter_context(tc.tile_pool(name="psum_mm", bufs=2, space="PSUM"))
        psum_tr = ctx.enter_context(tc.tile_pool(name="psum_tr", bufs=2, space="PSUM"))

        nc.gpsimd.load_library(library_config.mlp)

        def cload(ap_dram, shape, dtype, tag):
            t = const.tile(shape, dtype, tag=tag)
            nc.sync.dma_start(t[:], ap_dram)
            return t

        wsb = {n: cload(wdram[n][:, :], [P, D], dt.float16, n) for n in wdram}
        bp1b = cload(bp1b_d[:, :], [P, D], dt.float32, "bp1b")
        bl1c = cload(bl1c_d[:, :], [P, 1], dt.float32, "bl1c")
        bp2b = cload(bp2b_d[:, :], [P, D], dt.float32, "bp2b")
        bl2b = cload(bl2b_d[:, :], [P, D], dt.float32, "bl2b")
        iota = cload(iota_d[:, :], [P, P], dt.float16, "iota")
        ident = cload(ident_d[:, :], [P, P], dt.float16, "ident")
        dloc_sb = cload(dloc_d[:, :], [P, nb * KT], dt.float16, "dloc")
        invd_sb = cload(invd_d[:, :], [P, nloc_pad], dt.float32, "invd")
        idx0_sb = cload(idx0_d[:, :], [P, nb * K0 * P // 16], dt.int16, "idx0")
        idx1_sb = cload(idx1_d[:, :], [P, nb * K1 * P // 16], dt.int16, "idx1")

        xT_sb = persist.tile([P, nloc_pad], dt.float16, tag="xT")
        h1T_sb = persist.tile([P, nloc_pad], dt.float16, tag="h1T")

        def _iter_body():
            # ---------------- Phase A: layer-1 projection of own rows ----------
            for b in range(nb):
                sl = slice(b * P, (b + 1) * P)
                xblk = work.tile([P, D], dt.float32, tag="xblk")
                nc.sync.dma_start(xblk[:], x_own[sl, :])
                xb16 = work.tile([P, D], dt.float16, tag="xb16")
                nc.vector.tensor_copy(xb16[:], xblk[:])
                xT_ps = psum_tr.tile([P, P], dt.float16)
                nc.tensor.transpose(xT_ps[:], xb16[:], ident[:])
                nc.vector.tensor_copy(xT_sb[:, sl], xT_ps[:])
                p_ps = psum_mm.tile([P, D], dt.float32, tag="mm")
                nc.tensor.matmul(p_ps[:], lhsT=xT_sb[:, sl], rhs=wsb["Wp1T"][:], start=True, stop=True)
                pb = work.tile([P, D], dt.float32, tag="pb")
                nc.vector.tensor_tensor(out=pb[:], in0=p_ps[:], in1=bp1b[:], op=OP.add)
                pr = outp.tile([P, D], dt.float16, tag="pr")
                nc.scalar.activation(pr[:], pb[:], AF.Relu)
                nc.sync.dma_start(h1own[sl, :], pr[:])

            nc.gpsimd.collective_compute(
                "AllGather", OP.bypass, replica_groups=groups_all,
                ins=[h1own[:, :]], outs=[table1[:, :]],
            )

            # ---------------- message+aggregate for one layer -------------------
            def agg_layer(table, root_sb, WlT, WrT, layer):
                for g in range(ngroups):
                    st0 = stage_p.tile([P, G * K0, D], dt.float16, tag="st0")
                    c0 = G * K0 * P // 16
                    nc.gpsimd.dma_gather(
                        st0[:], table[0:nhalf, :], idx0_sb[:, g * c0 : (g + 1) * c0],
                        G * K0 * P, G * K0 * P, D, single_packet=False,
                    )
                    st1 = stage_p.tile([P, G * K1, D], dt.float16, tag="st1")
                    c1 = G * K1 * P // 16
                    nc.gpsimd.dma_gather(
                        st1[:], table[nhalf:npad, :], idx1_sb[:, g * c1 : (g + 1) * c1],
                        G * K1 * P, G * K1 * P, D, single_packet=False,
                    )
                    for bb in range(G):
                        b = g * G + bb
                        sl = slice(b * P, (b + 1) * P)
                        agg_ps = psum_agg.tile([P, P], dt.float32)
                        for t in range(KT):
                            oh = ohp.tile([P, P], dt.float16)
                            nc.vector.tensor_tensor(
                                out=oh[:],
                                in0=dloc_sb[:, b * KT + t : b * KT + t + 1].to_broadcast([P, P]),
                                in1=iota[:],
                                op=OP.is_equal,
                            )
                            msg = st0[:, bb * K0 + t, :] if t < K0 else st1[:, bb * K1 + (t - K0), :]
                            nc.tensor.matmul(
                                agg_ps[:], lhsT=msg, rhs=oh[:],
                                start=(t == 0), stop=(t == KT - 1),
                            )
                        aggT = aggsb.tile([P, P], dt.float16)
                        nc.vector.tensor_tensor(
                            out=aggT[:], in0=agg_ps[:], in1=invd_sb[:, sl], op=OP.mult
                        )
                        if layer == 1:
                            o_ps = psum_mm.tile([P, P], dt.float32, tag="mm")
                            nc.tensor.matmul(o_ps[:], lhsT=WlT[:], rhs=aggT[:], start=True, stop=False)
                            nc.tensor.matmul(o_ps[:], lhsT=WrT[:], rhs=root_sb[:, sl], start=False, stop=True)
                            nc.scalar.activation(h1T_sb[:, sl], o_ps[:], AF.Relu, bias=bl1c[:], scale=1.0)
                        else:
                            o_ps = psum_mm.tile([P, D], dt.float32, tag="mm")
                            nc.tensor.matmul(o_ps[:], lhsT=aggT[:], rhs=WlT[:], start=True, stop=False)
                            nc.tensor.matmul(o_ps[:], lhsT=root_sb[:, sl], rhs=WrT[:], start=False, stop=True)
                            ob = outp.tile([P, D], dt.float32, tag="ob")
                            nc.vector.tensor_tensor(out=ob[:], in0=o_ps[:], in1=bl2b[:], op=OP.add)
                            nc.sync.dma_start(out_own[sl, :], ob[:])

            # ---------------- Phase B: layer-1 aggregate -> h1T -----------------
            agg_layer(table1, xT_sb, wsb["Wl1T"], wsb["Wr1T"], layer=1)

            # ---------------- Phase C: layer-2 projection ----------------------
            for b in range(nb):
                sl = slice(b * P, (b + 1) * P)
                p_ps = psum_mm.tile([P, D], dt.float32, tag="mm")
                nc.tensor.matmul(p_ps[:], lhsT=h1T_sb[:, sl], rhs=wsb["Wp2T"][:], start=True, stop=True)
                pb = work.tile([P, D], dt.float32, tag="pb")
                nc.vector.tensor_tensor(out=pb[:], in0=p_ps[:], in1=bp2b[:], op=OP.add)
                pr = outp.tile([P, D], dt.float16, tag="pr")
                nc.scalar.activation(pr[:], pb[:], AF.Relu)
                nc.sync.dma_start(h2own[sl, :], pr[:])

            nc.gpsimd.collective_compute(
                "AllGather", OP.bypass, replica_groups=groups_all,
                ins=[h2own[:, :]], outs=[table2[:, :]],
            )

            # ---------------- Phase D: layer-2 aggregate -> out ----------------
            agg_layer(table2, h1T_sb, wsb["Wl2T"], wsb["Wr2T"], layer=2)

        for _ in range(iters):
            _iter_body()

    nc.compile()
    return nc


def make_in_maps(inputs, per_core, n_nodes, cores):
    nloc, nb, nloc_pad, npad, nhalf = _plan(n_nodes, cores)
    x = np.asarray(inputs["x"], dtype=np.float32)
    consts = dict(
        Wp1T=np.asarray(inputs["Wp1"]).T.astype(np.float16),
        Wl1T=np.asarray(inputs["Wl1"]).T.astype(np.float16),
        Wr1T=np.asarray(inputs["Wr1"]).T.astype(np.float16),
        Wp2T=np.asarray(inputs["Wp2"]).T.astype(np.float16),
        Wl2T=np.asarray(inputs["Wl2"]).T.astype(np.float16),
        Wr2T=np.asarray(inputs["Wr2"]).T.astype(np.float16),
        bp1b=np.broadcast_to(np.asarray(inputs["bp1"], np.float32)[None, :], (P, D)).copy(),
        bl1c=np.asarray(inputs["bl1"], np.float32).reshape(P, 1).copy(),
        bp2b=np.broadcast_to(np.asarray(inputs["bp2"], np.float32)[None, :], (P, D)).copy(),
        bl2b=np.broadcast_to(np.asarray(inputs["bl2"], np.float32)[None, :], (P, D)).copy(),
        iota=np.broadcast_to(np.arange(P, dtype=np.float16)[None, :], (P, P)).copy(),
        ident=np.eye(P, dtype=np.float16),
    )
    in_maps = []
    for c in range(cores):
        xo = np.zeros((nloc_pad, D), dtype=np.float32)
        xo[:nloc] = x[c * nloc : (c + 1) * nloc]
        m = dict(consts)
        m["x_own"] = xo
        m.update(per_core[c])
        in_maps.append(m)
    return in_maps


_BUILT = {}


def _run(inputs, n_nodes, n_edges, cores, G, trace=False):
    per_core, K0, K1, _ = preprocess(inputs["edge_index"], n_nodes, cores)
    key = (n_nodes, cores, K0, K1, G)
    if key not in _BUILT:
        _BUILT[key] = build_nc(n_nodes, cores, K0, K1, G)
    nc = _BUILT[key]
    in_maps = make_in_maps(inputs, per_core, n_nodes, cores)
    res = run_bass_kernel_spmd(nc, in_maps, list(range(cores)), trace=trace)
    nloc, nb, nloc_pad, npad, nhalf = _plan(n_nodes, cores)
    out = np.concatenate([res.results[c]["out_own"][:nloc] for c in range(cores)], axis=0)
    return out.astype(np.float32), res


def kernel(**inputs):
    out, _ = _run(inputs, N_NODES, N_EDGES, CORES, G=7)
    return out

